# revision 7
# baseline (speedup 1.0000x reference)
"""Trainium2 Bass kernel for nn_EqStftPBC (STFT perturbation-based compensation).

Math (per batch b, mode m; all shards independent):
  X = STFT(x)  (T=51 frames, F=80, hop=40, rect window)
  C_n2[g,t] = X[g,t]*conj(X[g-n2,t]) + (same at t-1, circular over T)
  U_n2 = M_n2.T @ C_n2      (circulant matmul == conv over n1 with w[:,n2])
  V_n2 = U_n2 * roll(X,n2)
  delta = sum_n2 V_n2 ;  d = ISTFT(delta) ; out = x + d*P  (+bias term)

Sharding: 8 cores = (b in 2) x (m in 2) x (n2-half in 2).  Each core computes
a partial delta over its 20 n2 values; host sums the two halves, adds x and
the bias term, slices.  All per-core variation is in the input data (S/M
matrix stacks), the program is uniform SPMD.

Layouts on device: everything is [frequency(80) partitions, time(51) free].
Rolled spectra R_n2 = roll(X, n2) are built with permutation matmuls
(S_n2.T @ X) so no cross-partition moves are needed.
"""

import numpy as np

import concourse.bass as bass
import concourse.bacc as bacc
import concourse.mybir as mybir
import concourse.tile as tile

F = 80
T = 51
HOP = 40
L = 2080
NJ = 20          # n2 values per core
NCH = 2          # chunks for pipelining
CHJ = NJ // NCH  # n2 per chunk
FP32 = mybir.dt.float32

# n2 lists per half (descending)
N2_LISTS = [list(range(19, -1, -1)), list(range(-1, -21, -1))]


def _dft_consts():
    j = np.arange(F)
    W = np.exp(-2j * np.pi * np.outer(j, j) / F)
    G = np.exp(+2j * np.pi * np.outer(j, j) / F) / F
    return W, G


def build_program(debug=False):
    nc = bacc.Bacc("TRN2", target_bir_lowering=False, debug=debug)

    xv = nc.dram_tensor("xv", [2, L], FP32, kind="ExternalInput")
    fr_c = nc.dram_tensor("fr_c", [2, F, F], FP32, kind="ExternalInput")   # Fr, Fi
    gr_c = nc.dram_tensor("gr_c", [2, F, F], FP32, kind="ExternalInput")   # Gr, Gi
    smat = nc.dram_tensor("smat", [NJ, F, F], FP32, kind="ExternalInput")
    mst = nc.dram_tensor("mst", [NJ, 2, F, F], FP32, kind="ExternalInput")
    svec = nc.dram_tensor("svec", [HOP, 52], FP32, kind="ExternalInput")
    yv = nc.dram_tensor("yv", [2, L], FP32, kind="ExternalOutput")

    with tile.TileContext(nc) as tc:
        with (
            tc.tile_pool(name="const", bufs=1) as cpool,
            tc.tile_pool(name="work", bufs=1) as wpool,
            tc.tile_pool(name="ps_x", bufs=1, space="PSUM") as ps_x,
            tc.tile_pool(name="ps_r", bufs=3, space="PSUM") as ps_r,
            tc.tile_pool(name="ps_u", bufs=3, space="PSUM") as ps_u,
            tc.tile_pool(name="ps_d", bufs=1, space="PSUM") as ps_d,
        ):
            # ---- load constants / inputs ----
            Fc = cpool.tile([F, 2 * F], FP32, tag="Fc")
            nc.sync.dma_start(Fc[:, 0:F], fr_c[0])
            nc.sync.dma_start(Fc[:, F:2 * F], fr_c[1])
            Gc = cpool.tile([F, 2 * F], FP32, tag="Gc")
            nc.sync.dma_start(Gc[:, 0:F], gr_c[0])
            nc.sync.dma_start(Gc[:, F:2 * F], gr_c[1])
            Ssb = cpool.tile([F, NJ * F], FP32, tag="Ssb")
            nc.sync.dma_start(
                Ssb[:, :],
                bass.AP(tensor=smat[:, :, :].tensor, offset=0,
                        ap=[[F, F], [F * F, NJ], [1, F]]),
            )
            Msb = cpool.tile([F, NJ * 2 * F], FP32, tag="Msb")
            nc.sync.dma_start(
                Msb[:, :],
                bass.AP(tensor=mst[:, :, :, :].tensor, offset=0,
                        ap=[[F, F], [2 * F * F, NJ], [F * F, 2], [1, F]]),
            )
            sv = cpool.tile([HOP, 52], FP32, tag="sv")
            nc.sync.dma_start(sv[:, :], svec[:, :])

            # frames [80j, (fiN | fr | fi)]
            frm = wpool.tile([F, 3 * T], FP32, tag="frm")
            nc.sync.dma_start(
                frm[:, T:2 * T],
                bass.AP(tensor=xv[:, :].tensor, offset=0, ap=[[1, F], [HOP, T]]),
            )
            nc.sync.dma_start(
                frm[:, 2 * T:3 * T],
                bass.AP(tensor=xv[:, :].tensor, offset=L, ap=[[1, F], [HOP, T]]),
            )
            nc.scalar.activation(frm[:, 0:T], frm[:, 2 * T:3 * T],
                                 mybir.ActivationFunctionType.Copy, scale=-1.0)

            # ---- STFT:  X = [Xr | Xi]  (f partitions, t free) ----
            Xp = ps_x.tile([F, 2 * T], FP32, tag="Xp")
            nc.tensor.matmul(Xp[:, :], Fc[:, 0:F], frm[:, T:3 * T], start=True, stop=False)
            nc.tensor.matmul(Xp[:, :], Fc[:, F:2 * F], frm[:, 0:2 * T], start=False, stop=True)
            Xsb = wpool.tile([F, 2 * T], FP32, tag="Xsb")
            nc.scalar.activation(Xsb[:, :], Xp[:, :], mybir.ActivationFunctionType.Copy)

            # ---- per-chunk pipeline ----
            Rsb, Csb, Usb, Vsb = [], [], [], []
            for c in range(NCH):
                Rsb.append(wpool.tile([F, CHJ * 2 * T], FP32, tag=f"Rsb{c}", name=f"Rsb{c}"))
                Csb.append(wpool.tile([F, CHJ * 3 * T], FP32, tag=f"Csb{c}", name=f"Csb{c}"))
                Usb.append(wpool.tile([F, CHJ * 2 * T], FP32, tag=f"Usb{c}", name=f"Usb{c}"))
                Vsb.append(wpool.tile([F, CHJ * 3 * T], FP32, tag=f"Vsb{c}", name=f"Vsb{c}"))
            sPR = wpool.tile([F, CHJ * T], FP32, tag="sPR")
            sPI = wpool.tile([F, CHJ * T], FP32, tag="sPI")
            sA = wpool.tile([F, CHJ * T], FP32, tag="sA")
            sB = wpool.tile([F, CHJ * T], FP32, tag="sB")
            sC = wpool.tile([F, CHJ * T], FP32, tag="sC")
            sD = wpool.tile([F, CHJ * T], FP32, tag="sD")

            Dp = ps_d.tile([F, 2 * T], FP32, tag="Dp")

            TT = nc.vector.tensor_tensor
            TG = nc.gpsimd.tensor_tensor
            MUL = mybir.AluOpType.mult
            ADD = mybir.AluOpType.add
            SUB = mybir.AluOpType.subtract

            PBK = 5  # matmul outputs packed per PSUM bank (5*102 <= 512)
            for c in range(NCH):
                # R build: permutation matmuls, pack 5 per bank, ACT-evict per bank
                for bk in range(CHJ // PBK):
                    Rp = ps_r.tile([F, PBK * 2 * T], FP32, tag="Rp")
                    for s in range(PBK):
                        j = c * CHJ + bk * PBK + s
                        nc.tensor.matmul(Rp[:, s * 2 * T:(s + 1) * 2 * T],
                                         Ssb[:, j * F:(j + 1) * F],
                                         Xsb[:, :], start=True, stop=True)
                    nc.scalar.activation(
                        Rsb[c][:, bk * PBK * 2 * T:(bk + 1) * PBK * 2 * T],
                        Rp[:, :], mybir.ActivationFunctionType.Copy)

                Rv = Rsb[c][:, :].rearrange("p (j s) -> p j s", j=CHJ)
                Rr = Rv[:, :, 0:T]
                Ri = Rv[:, :, T:2 * T]
                Xbr = Xsb[:, None, 0:T].to_broadcast([F, CHJ, T])
                Xbi = Xsb[:, None, T:2 * T].to_broadcast([F, CHJ, T])
                vPR = sPR[:, :].rearrange("p (j t) -> p j t", j=CHJ)
                vPI = sPI[:, :].rearrange("p (j t) -> p j t", j=CHJ)
                vA = sA[:, :].rearrange("p (j t) -> p j t", j=CHJ)
                vB = sB[:, :].rearrange("p (j t) -> p j t", j=CHJ)
                vC = sC[:, :].rearrange("p (j t) -> p j t", j=CHJ)
                vD = sD[:, :].rearrange("p (j t) -> p j t", j=CHJ)

                # C_pre = X * conj(R):  re = Xr*Rr + Xi*Ri ; im = Xi*Rr - Xr*Ri
                TT(vA, Xbr, Rr, MUL)
                TT(vB, Xbi, Ri, MUL)
                TT(vPR, vA, vB, ADD)
                TG(vC, Xbi, Rr, MUL)
                TG(vD, Xbr, Ri, MUL)
                TG(vPI, vC, vD, SUB)

                # C = C_pre + roll_t(C_pre), into slots [CiN | Cr | Ci]
                Cv = Csb[c][:, :].rearrange("p (j s) -> p j s", j=CHJ)
                TT(Cv[:, :, T + 1:2 * T], vPR[:, :, 1:T], vPR[:, :, 0:T - 1], ADD)
                TT(Cv[:, :, T:T + 1], vPR[:, :, 0:1], vPR[:, :, T - 1:T], ADD)
                TG(Cv[:, :, 2 * T + 1:3 * T], vPI[:, :, 1:T], vPI[:, :, 0:T - 1], ADD)
                TG(Cv[:, :, 2 * T:2 * T + 1], vPI[:, :, 0:1], vPI[:, :, T - 1:T], ADD)
                nc.scalar.activation(Cv[:, :, 0:T], Cv[:, :, 2 * T:3 * T],
                                     mybir.ActivationFunctionType.Copy, scale=-1.0)

                # stage-1: U = M.T @ C  -> [Ur | Ui], pack 5 per bank, ACT-evict
                for bk in range(CHJ // PBK):
                    Up = ps_u.tile([F, PBK * 2 * T], FP32, tag="Up")
                    for s in range(PBK):
                        jj = bk * PBK + s
                        j = c * CHJ + jj
                        nc.tensor.matmul(Up[:, s * 2 * T:(s + 1) * 2 * T],
                                         Msb[:, (2 * j) * F:(2 * j + 1) * F],
                                         Cv[:, jj, T:3 * T], start=True, stop=False)
                        nc.tensor.matmul(Up[:, s * 2 * T:(s + 1) * 2 * T],
                                         Msb[:, (2 * j + 1) * F:(2 * j + 2) * F],
                                         Cv[:, jj, 0:2 * T], start=False, stop=True)
                    nc.scalar.activation(
                        Usb[c][:, bk * PBK * 2 * T:(bk + 1) * PBK * 2 * T],
                        Up[:, :], mybir.ActivationFunctionType.Copy)

                # stage-2: V = U * R -> slots [ViN | Vr | Vi]
                Uv = Usb[c][:, :].rearrange("p (j s) -> p j s", j=CHJ)
                Ur = Uv[:, :, 0:T]
                Ui = Uv[:, :, T:2 * T]
                Vv = Vsb[c][:, :].rearrange("p (j s) -> p j s", j=CHJ)
                TT(vA, Ur, Rr, MUL)
                TT(vB, Ui, Ri, MUL)
                TT(Vv[:, :, T:2 * T], vA, vB, SUB)
                TG(vC, Ur, Ri, MUL)
                TG(vD, Ui, Rr, MUL)
                TG(Vv[:, :, 2 * T:3 * T], vC, vD, ADD)
                nc.scalar.activation(Vv[:, :, 0:T], Vv[:, :, 2 * T:3 * T],
                                     mybir.ActivationFunctionType.Copy, scale=-1.0)

                # final: accumulate D += Gr.T@[Vr|Vi] + Gi.T@[ViN|Vr]
                for jj in range(CHJ):
                    j = c * CHJ + jj
                    nc.tensor.matmul(Dp[:, :], Gc[:, 0:F], Vv[:, jj, T:3 * T],
                                     start=(j == 0), stop=False)
                    nc.tensor.matmul(Dp[:, :], Gc[:, F:2 * F], Vv[:, jj, 0:2 * T],
                                     start=False, stop=(j == NJ - 1))

            # ---- overlap-add + scale ----
            Dsb = wpool.tile([F, 2 * T], FP32, tag="Dsb")
            nc.scalar.activation(Dsb[:, :], Dp[:, :],
                                 mybir.ActivationFunctionType.Copy)
            S2 = wpool.tile([HOP, 2 * T], FP32, tag="S2")
            nc.sync.dma_start(S2[:, :], Dsb[HOP:F, :])
            Y = wpool.tile([HOP, 2 * 52], FP32, tag="Y")
            S1v = Dsb[0:HOP, :].rearrange("p (c t) -> p c t", c=2)
            S2v = S2[:, :].rearrange("p (c t) -> p c t", c=2)
            Yv = Y[:, :].rearrange("p (c t) -> p c t", c=2)
            TT(Yv[:, :, 1:T], S1v[:, :, 1:T], S2v[:, :, 0:T - 1], ADD)
            nc.scalar.activation(Yv[:, :, 0:1], S1v[:, :, 0:1],
                                 mybir.ActivationFunctionType.Copy)
            nc.scalar.activation(Yv[:, :, T:52], S2v[:, :, T - 1:T],
                                 mybir.ActivationFunctionType.Copy)
            TT(Yv, Yv, sv[:, None, :].to_broadcast([HOP, 2, 52]), MUL)

            # out DMA: y[l] with l = 40*t' + tau'
            for p in range(2):
                nc.sync.dma_start(
                    bass.AP(tensor=yv[:, :].tensor, offset=p * L,
                            ap=[[1, HOP], [HOP, 52]]),
                    Y[:, p * 52:(p + 1) * 52],
                )
    return nc


# ---------------- host side ----------------

def _host_consts():
    W, G = _dft_consts()
    fr_c = np.stack([W.real, W.imag]).astype(np.float32)
    gr_c = np.stack([G.real, G.imag]).astype(np.float32)
    cov = np.zeros(L)
    idx = (np.arange(T)[:, None] * HOP + np.arange(F)[None, :]).reshape(-1)
    np.add.at(cov, idx, 1.0)
    cov = np.where(cov > 0, cov, 1.0)
    return fr_c, gr_c, cov


def _smat_for(n2_list):
    S = np.zeros((NJ, F, F), np.float32)
    g = np.arange(F)
    for j, n2 in enumerate(n2_list):
        S[j, (g - n2) % F, g] = 1.0
    return S


def _mst_for(n2_list, w2):
    # M_n2[g,f] = w2[n1+20, n2+20] with n1 = ((f-g+20)%80)-20 if in [-20,19]
    Ms = np.zeros((NJ, 2, F, F), np.float32)
    g = np.arange(F)[:, None]
    f = np.arange(F)[None, :]
    n1 = ((f - g + 20) % F) - 20
    valid = (n1 >= -20) & (n1 <= 19)
    n1c = np.clip(n1 + 20, 0, 39)
    for j, n2 in enumerate(n2_list):
        col = w2[:, n2 + 20]
        Mr = np.where(valid, col.real[n1c], 0.0)
        Mi = np.where(valid, col.imag[n1c], 0.0)
        Ms[j, 0] = Mr
        Ms[j, 1] = Mi
    return Ms


def make_in_maps(x_real, x_imag, task_info, w_real, w_imag):
    fr_c, gr_c, cov = _host_consts()
    b, _, m = x_real.shape
    P = np.power(10.0, task_info[:, 0] / 10.0) / m
    w2 = (np.asarray(w_real) + 1j * np.asarray(w_imag)).reshape(40, 40)
    smats = [_smat_for(nl) for nl in N2_LISTS]
    msts = [_mst_for(nl, w2) for nl in N2_LISTS]

    in_maps = []
    shards = []
    for bb in range(b):
        for mm in range(m):
            for h in range(2):
                xvv = np.stack([x_real[bb, :, mm], x_imag[bb, :, mm]]).astype(np.float32)
                sv = np.zeros((HOP, 52), np.float32)
                for tp in range(52):
                    for tau in range(HOP):
                        l = HOP * tp + tau
                        if l < L:
                            sv[tau, tp] = P[bb] / cov[l]
                in_maps.append({
                    "xv": xvv,
                    "fr_c": fr_c,
                    "gr_c": gr_c,
                    "smat": smats[h],
                    "mst": msts[h].reshape(NJ, 2, F, F),
                    "svec": sv,
                })
                shards.append((bb, mm, h))
    return in_maps, shards, P, cov


_NC_CACHE = {}


def kernel(x_real, x_imag, task_info, w_real, w_imag, b_real, b_imag):
    x_real = np.asarray(x_real)
    x_imag = np.asarray(x_imag)
    task_info = np.asarray(task_info)
    b, Lx, m = x_real.shape
    assert (b, Lx, m) == (2, L, 2)

    if "nc" not in _NC_CACHE:
        nc_ = build_program(debug=False)
        nc_.compile()
        _NC_CACHE["nc"] = nc_
    nc = _NC_CACHE["nc"]

    in_maps, shards, P, cov = make_in_maps(x_real, x_imag, task_info, w_real, w_imag)
    from concourse.bass_utils import run_bass_kernel_spmd
    res = run_bass_kernel_spmd(nc, in_maps, list(range(8))).results

    x = (x_real + 1j * x_imag).astype(np.complex64)
    out = x.copy()
    bias = complex(np.asarray(b_real)[0], np.asarray(b_imag)[0])
    # bias ISTFT term: delta const over f -> impulse at tau=0 of each frame
    bias_sig = np.zeros(L, np.complex64)
    bias_sig[np.arange(T) * HOP] = bias
    bias_sig /= cov
    for i, (bb, mm, h) in enumerate(shards):
        yvv = res[i]["yv"]
        out[bb, :, mm] += yvv[0] + 1j * yvv[1]
    for bb in range(b):
        for mm in range(m):
            out[bb, :, mm] += (P[bb] * bias_sig).astype(np.complex64)
    return out[:, 20:L - 20, :]


# revision 10
# speedup vs baseline: 1.3552x; 1.3552x over previous
"""Trainium2 Bass kernel for nn_EqStftPBC (STFT perturbation-based compensation).

Per (batch b, mode m) — all independent:
  X = STFT(x); C_n2 = X*conj(roll(X,n2)) + prev-frame; U_n2 = circulant(w[:,n2]) conv;
  V_n2 = U_n2 * roll(X,n2); delta = sum_n2 V_n2; out = x + ISTFT(delta)*P (+bias)

8 cores = (b x m x n2-half).  Uniform SPMD program; all per-core variation is in
input data (permutation stacks S, circulant stacks M).  Device layout: [freq(80)
partitions, time(51) free].  STFT in fp32; everything downstream bf16 (PSUM
accumulation fp32).
"""

import numpy as np
from ml_dtypes import bfloat16

import concourse.bass as bass
import concourse.bacc as bacc
import concourse.mybir as mybir
import concourse.tile as tile

F = 80
T = 51
TP = 52          # padded slot stride (even -> 4B-aligned bf16 slots)
HOP = 40
L = 2080
NJ = 20          # n2 values per core
NCH = 2          # chunks for pipelining
CHJ = NJ // NCH  # n2 per chunk
PBK = 5          # matmul outputs packed per PSUM bank (5*102 <= 512)
FP32 = mybir.dt.float32
BF16 = mybir.dt.bfloat16

N2_LISTS = [list(range(19, -1, -1)), list(range(-1, -21, -1))]


def _dft_consts():
    j = np.arange(F)
    W = np.exp(-2j * np.pi * np.outer(j, j) / F)
    G = np.exp(+2j * np.pi * np.outer(j, j) / F) / F
    return W, G


def build_program(debug=False):
    nc = bacc.Bacc("TRN2", target_bir_lowering=False, debug=debug)

    xv = nc.dram_tensor("xv", [2, L], FP32, kind="ExternalInput")
    fr_c = nc.dram_tensor("fr_c", [2, F, F], FP32, kind="ExternalInput")
    gr_c = nc.dram_tensor("gr_c", [2, F, F], BF16, kind="ExternalInput")
    smat = nc.dram_tensor("smat", [NJ, F, F], BF16, kind="ExternalInput")
    mst = nc.dram_tensor("mst", [NJ, 2, F, F], BF16, kind="ExternalInput")
    svec = nc.dram_tensor("svec", [HOP, 52], FP32, kind="ExternalInput")
    yv = nc.dram_tensor("yv", [2, L], FP32, kind="ExternalOutput")

    with tile.TileContext(nc) as tc:
        with (
            tc.tile_pool(name="const", bufs=1) as cpool,
            tc.tile_pool(name="work", bufs=1) as wpool,
            tc.tile_pool(name="ps_x", bufs=1, space="PSUM") as ps_x,
            tc.tile_pool(name="ps_r", bufs=2, space="PSUM") as ps_r,
            tc.tile_pool(name="ps_u", bufs=2, space="PSUM") as ps_u,
            tc.tile_pool(name="ps_d", bufs=1, space="PSUM") as ps_d,
        ):
            # ---- constants ----
            Fc = cpool.tile([F, 2 * F], FP32, tag="Fc")
            nc.sync.dma_start(Fc[:, 0:F], fr_c[0])
            nc.sync.dma_start(Fc[:, F:2 * F], fr_c[1])
            Gc = cpool.tile([F, 2 * F], BF16, tag="Gc")
            nc.sync.dma_start(Gc[:, 0:F], gr_c[0])
            nc.sync.dma_start(Gc[:, F:2 * F], gr_c[1])
            Ssb = cpool.tile([F, NJ * F], BF16, tag="Ssb")
            nc.sync.dma_start(
                Ssb[:, :],
                bass.AP(tensor=smat[:, :, :].tensor, offset=0,
                        ap=[[F, F], [F * F, NJ], [1, F]]),
            )
            Msb = cpool.tile([F, NJ * 2 * F], BF16, tag="Msb")
            nc.sync.dma_start(
                Msb[:, :],
                bass.AP(tensor=mst[:, :, :, :].tensor, offset=0,
                        ap=[[F, F], [2 * F * F, NJ], [F * F, 2], [1, F]]),
            )
            sv = cpool.tile([HOP, 52], FP32, tag="sv")
            nc.sync.dma_start(sv[:, :], svec[:, :])

            # frames [80j, (fiN | fr | fi)]  fp32
            frm = wpool.tile([F, 3 * T], FP32, tag="frm")
            nc.sync.dma_start(
                frm[:, T:2 * T],
                bass.AP(tensor=xv[:, :].tensor, offset=0, ap=[[1, F], [HOP, T]]),
            )
            nc.sync.dma_start(
                frm[:, 2 * T:3 * T],
                bass.AP(tensor=xv[:, :].tensor, offset=L, ap=[[1, F], [HOP, T]]),
            )
            nc.scalar.activation(frm[:, 0:T], frm[:, 2 * T:3 * T],
                                 mybir.ActivationFunctionType.Copy, scale=-1.0)

            # ---- STFT (fp32):  X = [Xr | Xi] -> evict to bf16 padded slots ----
            Xp = ps_x.tile([F, 2 * T], FP32, tag="Xp")
            nc.tensor.matmul(Xp[:, :], Fc[:, 0:F], frm[:, T:3 * T], start=True, stop=False)
            nc.tensor.matmul(Xp[:, :], Fc[:, F:2 * F], frm[:, 0:2 * T], start=False, stop=True)
            Xsb = wpool.tile([F, 2 * TP], BF16, tag="Xsb")
            Xsv = Xsb[:, :].rearrange("p (c t) -> p c t", c=2)
            Xpv = Xp[:, :].rearrange("p (c t) -> p c t", c=2)
            nc.scalar.activation(Xsv[:, :, 0:T], Xpv[:, :, 0:T],
                                 mybir.ActivationFunctionType.Copy)

            # ---- per-chunk tiles ----
            Rsb, Csb, Usb, Vsb = [], [], [], []
            for c in range(NCH):
                Rsb.append(wpool.tile([F, CHJ * 2 * TP], BF16, tag=f"Rsb{c}", name=f"Rsb{c}"))
                Csb.append(wpool.tile([F, CHJ * 3 * TP], BF16, tag=f"Csb{c}", name=f"Csb{c}"))
                Usb.append(wpool.tile([F, CHJ * 2 * TP], BF16, tag=f"Usb{c}", name=f"Usb{c}"))
                Vsb.append(wpool.tile([F, CHJ * 3 * TP], BF16, tag=f"Vsb{c}", name=f"Vsb{c}"))
            sPR = wpool.tile([F, CHJ * TP], BF16, tag="sPR")
            sPI = wpool.tile([F, CHJ * TP], BF16, tag="sPI")
            sA = wpool.tile([F, CHJ * TP], BF16, tag="sA")
            sB = wpool.tile([F, CHJ * TP], BF16, tag="sB")
            sC = wpool.tile([F, CHJ * TP], BF16, tag="sC")
            sD = wpool.tile([F, CHJ * TP], BF16, tag="sD")

            Dp = ps_d.tile([F, 2 * T], FP32, tag="Dp")

            TT = nc.vector.tensor_tensor
            TG = nc.gpsimd.tensor_tensor
            MUL = mybir.AluOpType.mult
            ADD = mybir.AluOpType.add
            SUB = mybir.AluOpType.subtract

            for c in range(NCH):
                # R build: permutation matmuls (bf16), 5 per bank, ACT-evict
                for bk in range(CHJ // PBK):
                    Rp = ps_r.tile([F, PBK * 2 * T], FP32, tag="Rp")
                    Xrhs = bass.AP(tensor=Xsb[:, :].tensor, offset=Xsb[:, :].offset,
                                   ap=[[2 * TP, F], [TP, 2], [1, T]])
                    for s in range(PBK):
                        j = c * CHJ + bk * PBK + s
                        nc.tensor.matmul(Rp[:, s * 2 * T:(s + 1) * 2 * T],
                                         Ssb[:, j * F:(j + 1) * F],
                                         Xrhs, start=True, stop=True)
                    # evict psum (5j x [Ur51|Ui51]) -> bf16 padded slots
                    nc.scalar.activation(
                        bass.AP(tensor=Rsb[c][:, :].tensor,
                                offset=Rsb[c][:, :].offset + bk * PBK * 2 * TP,
                                ap=[[CHJ * 2 * TP, F], [2 * TP, PBK], [TP, 2], [1, T]]),
                        Rp[:, :].rearrange("p (s c t) -> p s c t", s=PBK, c=2),
                        mybir.ActivationFunctionType.Copy)

                Rv = Rsb[c][:, :].rearrange("p (j s) -> p j s", j=CHJ)
                Rr = Rv[:, :, 0:T]
                Ri = Rv[:, :, TP:TP + T]
                Xbr = Xsb[:, None, 0:T].to_broadcast([F, CHJ, T])
                Xbi = Xsb[:, None, TP:TP + T].to_broadcast([F, CHJ, T])
                vPR = sPR[:, :].rearrange("p (j t) -> p j t", j=CHJ)[:, :, 0:T]
                vPI = sPI[:, :].rearrange("p (j t) -> p j t", j=CHJ)[:, :, 0:T]
                vA = sA[:, :].rearrange("p (j t) -> p j t", j=CHJ)[:, :, 0:T]
                vB = sB[:, :].rearrange("p (j t) -> p j t", j=CHJ)[:, :, 0:T]
                vC = sC[:, :].rearrange("p (j t) -> p j t", j=CHJ)[:, :, 0:T]
                vD = sD[:, :].rearrange("p (j t) -> p j t", j=CHJ)[:, :, 0:T]

                # C_pre = X * conj(R)
                TT(vA, Xbr, Rr, MUL)
                TT(vB, Xbi, Ri, MUL)
                TT(vPR, vA, vB, ADD)
                TG(vC, Xbi, Rr, MUL)
                TG(vD, Xbr, Ri, MUL)
                TG(vPI, vC, vD, SUB)

                # C = C_pre + roll_t(C_pre), slots [CiN | Cr | Ci]
                Cv = Csb[c][:, :].rearrange("p (j s) -> p j s", j=CHJ)
                TT(Cv[:, :, TP + 1:TP + T], vPR[:, :, 1:T], vPR[:, :, 0:T - 1], ADD)
                TT(Cv[:, :, TP:TP + 1], vPR[:, :, 0:1], vPR[:, :, T - 1:T], ADD)
                TG(Cv[:, :, 2 * TP + 1:2 * TP + T], vPI[:, :, 1:T], vPI[:, :, 0:T - 1], ADD)
                TG(Cv[:, :, 2 * TP:2 * TP + 1], vPI[:, :, 0:1], vPI[:, :, T - 1:T], ADD)
                nc.scalar.activation(Cv[:, :, 0:T], Cv[:, :, 2 * TP:2 * TP + T],
                                     mybir.ActivationFunctionType.Copy, scale=-1.0)

                # stage-1: U = M.T @ C -> [Ur | Ui] (psum packed), ACT-evict
                for bk in range(CHJ // PBK):
                    Up = ps_u.tile([F, PBK * 2 * T], FP32, tag="Up")
                    for s in range(PBK):
                        jj = bk * PBK + s
                        j = c * CHJ + jj
                        rhs1 = bass.AP(tensor=Csb[c][:, :].tensor,
                                       offset=Csb[c][:, :].offset + jj * 3 * TP + TP,
                                       ap=[[3 * CHJ * TP, F], [TP, 2], [1, T]])
                        rhs2 = bass.AP(tensor=Csb[c][:, :].tensor,
                                       offset=Csb[c][:, :].offset + jj * 3 * TP,
                                       ap=[[3 * CHJ * TP, F], [TP, 2], [1, T]])
                        nc.tensor.matmul(Up[:, s * 2 * T:(s + 1) * 2 * T],
                                         Msb[:, (2 * j) * F:(2 * j + 1) * F],
                                         rhs1, start=True, stop=False)
                        nc.tensor.matmul(Up[:, s * 2 * T:(s + 1) * 2 * T],
                                         Msb[:, (2 * j + 1) * F:(2 * j + 2) * F],
                                         rhs2, start=False, stop=True)
                    nc.scalar.activation(
                        bass.AP(tensor=Usb[c][:, :].tensor,
                                offset=Usb[c][:, :].offset + bk * PBK * 2 * TP,
                                ap=[[CHJ * 2 * TP, F], [2 * TP, PBK], [TP, 2], [1, T]]),
                        Up[:, :].rearrange("p (s c t) -> p s c t", s=PBK, c=2),
                        mybir.ActivationFunctionType.Copy)

                # stage-2: V = U * R, slots [ViN | Vr | Vi]
                Uv = Usb[c][:, :].rearrange("p (j s) -> p j s", j=CHJ)
                Ur = Uv[:, :, 0:T]
                Ui = Uv[:, :, TP:TP + T]
                Vv = Vsb[c][:, :].rearrange("p (j s) -> p j s", j=CHJ)
                TT(vA, Ur, Rr, MUL)
                TT(vB, Ui, Ri, MUL)
                TT(Vv[:, :, TP:TP + T], vA, vB, SUB)
                TG(vC, Ur, Ri, MUL)
                TG(vD, Ui, Rr, MUL)
                TG(Vv[:, :, 2 * TP:2 * TP + T], vC, vD, ADD)
                nc.scalar.activation(Vv[:, :, 0:T], Vv[:, :, 2 * TP:2 * TP + T],
                                     mybir.ActivationFunctionType.Copy, scale=-1.0)

                # final: D += Gr.T@[Vr|Vi] (all j), then Gi.T@[ViN|Vr] (all j)
                for gpass in range(2):
                    for jj in range(CHJ):
                        j = c * CHJ + jj
                        off = jj * 3 * TP + (TP if gpass == 0 else 0)
                        rhs = bass.AP(tensor=Vsb[c][:, :].tensor,
                                      offset=Vsb[c][:, :].offset + off,
                                      ap=[[3 * CHJ * TP, F], [TP, 2], [1, T]])
                        nc.tensor.matmul(Dp[:, :], Gc[:, gpass * F:(gpass + 1) * F],
                                         rhs, start=(c == 0 and gpass == 0 and jj == 0),
                                         stop=(c == NCH - 1 and gpass == 1 and jj == CHJ - 1))

            # ---- overlap-add + scale (fp32) ----
            Dsb = wpool.tile([F, 2 * T], FP32, tag="Dsb")
            nc.scalar.activation(Dsb[:, :], Dp[:, :],
                                 mybir.ActivationFunctionType.Copy)
            S2 = wpool.tile([HOP, 2 * T], FP32, tag="S2")
            nc.sync.dma_start(S2[:, :], Dsb[HOP:F, :])
            Y = wpool.tile([HOP, 2 * 52], FP32, tag="Y")
            S1v = Dsb[0:HOP, :].rearrange("p (c t) -> p c t", c=2)
            S2v = S2[:, :].rearrange("p (c t) -> p c t", c=2)
            Yv = Y[:, :].rearrange("p (c t) -> p c t", c=2)
            TT(Yv[:, :, 1:T], S1v[:, :, 1:T], S2v[:, :, 0:T - 1], ADD)
            nc.scalar.activation(Yv[:, :, 0:1], S1v[:, :, 0:1],
                                 mybir.ActivationFunctionType.Copy)
            nc.scalar.activation(Yv[:, :, T:52], S2v[:, :, T - 1:T],
                                 mybir.ActivationFunctionType.Copy)
            TT(Yv, Yv, sv[:, None, :].to_broadcast([HOP, 2, 52]), MUL)

            for p in range(2):
                nc.sync.dma_start(
                    bass.AP(tensor=yv[:, :].tensor, offset=p * L,
                            ap=[[1, HOP], [HOP, 52]]),
                    Y[:, p * 52:(p + 1) * 52],
                )
    return nc


# ---------------- host side ----------------

def _host_consts():
    W, G = _dft_consts()
    fr_c = np.stack([W.real, W.imag]).astype(np.float32)
    gr_c = np.stack([G.real, G.imag]).astype(bfloat16)
    cov = np.zeros(L)
    idx = (np.arange(T)[:, None] * HOP + np.arange(F)[None, :]).reshape(-1)
    np.add.at(cov, idx, 1.0)
    cov = np.where(cov > 0, cov, 1.0)
    return fr_c, gr_c, cov


def _smat_for(n2_list):
    S = np.zeros((NJ, F, F), np.float32)
    g = np.arange(F)
    for j, n2 in enumerate(n2_list):
        S[j, (g - n2) % F, g] = 1.0
    return S.astype(bfloat16)


def _mst_for(n2_list, w2):
    Ms = np.zeros((NJ, 2, F, F), np.float32)
    g = np.arange(F)[:, None]
    f = np.arange(F)[None, :]
    n1 = ((f - g + 20) % F) - 20
    valid = (n1 >= -20) & (n1 <= 19)
    n1c = np.clip(n1 + 20, 0, 39)
    for j, n2 in enumerate(n2_list):
        col = w2[:, n2 + 20]
        Ms[j, 0] = np.where(valid, col.real[n1c], 0.0)
        Ms[j, 1] = np.where(valid, col.imag[n1c], 0.0)
    return Ms.astype(bfloat16)


def make_in_maps(x_real, x_imag, task_info, w_real, w_imag):
    fr_c, gr_c, cov = _host_consts()
    b, _, m = x_real.shape
    P = np.power(10.0, task_info[:, 0] / 10.0) / m
    w2 = (np.asarray(w_real) + 1j * np.asarray(w_imag)).reshape(40, 40)
    smats = [_smat_for(nl) for nl in N2_LISTS]
    msts = [_mst_for(nl, w2) for nl in N2_LISTS]

    svs = []
    tp = np.arange(52)[None, :]
    tau = np.arange(HOP)[:, None]
    l = HOP * tp + tau  # always < L
    for bb in range(b):
        svs.append((P[bb] / cov[l]).astype(np.float32))

    in_maps = []
    shards = []
    for bb in range(b):
        for mm in range(m):
            for h in range(2):
                xvv = np.stack([x_real[bb, :, mm], x_imag[bb, :, mm]]).astype(np.float32)
                in_maps.append({
                    "xv": xvv,
                    "fr_c": fr_c,
                    "gr_c": gr_c,
                    "smat": smats[h],
                    "mst": msts[h],
                    "svec": svs[bb],
                })
                shards.append((bb, mm, h))
    return in_maps, shards, P, cov


_NC_CACHE = {}


def kernel(x_real, x_imag, task_info, w_real, w_imag, b_real, b_imag):
    x_real = np.asarray(x_real)
    x_imag = np.asarray(x_imag)
    task_info = np.asarray(task_info)
    b, Lx, m = x_real.shape
    assert (b, Lx, m) == (2, L, 2)

    if "nc" not in _NC_CACHE:
        nc_ = build_program(debug=False)
        nc_.compile()
        _NC_CACHE["nc"] = nc_
    nc = _NC_CACHE["nc"]

    in_maps, shards, P, cov = make_in_maps(x_real, x_imag, task_info, w_real, w_imag)
    from concourse.bass_utils import run_bass_kernel_spmd
    res = run_bass_kernel_spmd(nc, in_maps, list(range(8))).results

    x = (x_real + 1j * x_imag).astype(np.complex64)
    out = x.copy()
    bias = complex(np.asarray(b_real)[0], np.asarray(b_imag)[0])
    bias_sig = np.zeros(L, np.complex64)
    bias_sig[np.arange(T) * HOP] = bias
    bias_sig /= cov
    for i, (bb, mm, h) in enumerate(shards):
        yvv = res[i]["yv"]
        out[bb, :, mm] += yvv[0] + 1j * yvv[1]
    for bb in range(b):
        for mm in range(m):
            out[bb, :, mm] += (P[bb] * bias_sig).astype(np.complex64)
    return out[:, 20:L - 20, :]


# revision 17
# speedup vs baseline: 1.4499x; 1.0699x over previous
"""Trainium2 Bass kernel for nn_EqStftPBC (STFT perturbation-based compensation).

Per (batch b, mode m) — all independent:
  X = STFT(x); C_n2 = X*conj(roll(X,n2)) + prev-frame; U_n2 = circulant(w[:,n2]) conv;
  V_n2 = U_n2 * roll(X,n2); delta = sum_n2 V_n2; out = x + ISTFT(delta)*P (+bias)

8 cores = (b x m x n2-half).  Uniform SPMD program; all per-core variation is in
input data (permutation stacks S, circulant stacks M).  Device layout: [freq(80)
partitions, time(51) free].  STFT in fp32; everything downstream bf16 (PSUM
accumulation fp32).
"""

import numpy as np
from ml_dtypes import bfloat16

import concourse.bass as bass
import concourse.bacc as bacc
import concourse.mybir as mybir
import concourse.tile as tile

F = 80
T = 51
TP = 52          # padded slot stride (even -> 4B-aligned bf16 slots)
HOP = 40
L = 2080
NJ = 20          # n2 values per core
NCH = 2          # chunks for pipelining
CHJ = NJ // NCH  # n2 per chunk
PBK = 5          # matmul outputs packed per PSUM bank (5*102 <= 512)
FP32 = mybir.dt.float32
BF16 = mybir.dt.bfloat16

N2_LISTS = [list(range(19, -1, -1)), list(range(-1, -21, -1))]


def _dft_consts():
    j = np.arange(F)
    W = np.exp(-2j * np.pi * np.outer(j, j) / F)
    G = np.exp(+2j * np.pi * np.outer(j, j) / F) / F
    return W, G


def build_program(debug=False):
    nc = bacc.Bacc("TRN2", target_bir_lowering=False, debug=debug)

    # all const tensors already in device layout [g, ...] for contiguous DMA
    xv = nc.dram_tensor("xv", [2, L], FP32, kind="ExternalInput")
    fr_c = nc.dram_tensor("fr_c", [F, 2 * F], FP32, kind="ExternalInput")
    gr_c = nc.dram_tensor("gr_c", [F, 2 * F], BF16, kind="ExternalInput")
    smat = nc.dram_tensor("smat", [F, NJ * F], BF16, kind="ExternalInput")
    mst = nc.dram_tensor("mst", [F, NJ * 2 * F], BF16, kind="ExternalInput")
    svec = nc.dram_tensor("svec", [HOP, 52], FP32, kind="ExternalInput")
    yv = nc.dram_tensor("yv", [2, L], FP32, kind="ExternalOutput")

    with tile.TileContext(nc) as tc:
        with (
            tc.tile_pool(name="const", bufs=1) as cpool,
            tc.tile_pool(name="work", bufs=1) as wpool,
            tc.tile_pool(name="ps_x", bufs=1, space="PSUM") as ps_x,
            tc.tile_pool(name="ps_r", bufs=2, space="PSUM") as ps_r,
            tc.tile_pool(name="ps_u", bufs=2, space="PSUM") as ps_u,
            tc.tile_pool(name="ps_d", bufs=1, space="PSUM") as ps_d,
        ):
            # frames [80j, (fiN | fr | fi)]  fp32  -- loaded FIRST
            frm = wpool.tile([F, 3 * T], FP32, tag="frm")
            nc.sync.dma_start(
                frm[:, T:2 * T],
                bass.AP(tensor=xv[:, :].tensor, offset=0, ap=[[1, F], [HOP, T]]),
            )
            nc.sync.dma_start(
                frm[:, 2 * T:3 * T],
                bass.AP(tensor=xv[:, :].tensor, offset=L, ap=[[1, F], [HOP, T]]),
            )
            # ---- constants (contiguous 2D DMAs) ----
            Fc = cpool.tile([F, 2 * F], FP32, tag="Fc")
            nc.sync.dma_start(Fc[:, :], fr_c[:, :])
            Ssb = cpool.tile([F, NJ * F], BF16, tag="Ssb")
            nc.sync.dma_start(Ssb[:, :], smat[:, :])
            Msb = cpool.tile([F, NJ * 2 * F], BF16, tag="Msb")
            nc.sync.dma_start(Msb[:, :], mst[:, :])
            Gc = cpool.tile([F, 2 * F], BF16, tag="Gc")
            nc.sync.dma_start(Gc[:, :], gr_c[:, :])
            sv = cpool.tile([HOP, 52], FP32, tag="sv")
            nc.sync.dma_start(sv[:, :], svec[:, :])
            nc.scalar.activation(frm[:, 0:T], frm[:, 2 * T:3 * T],
                                 mybir.ActivationFunctionType.Copy, scale=-1.0)

            # ---- STFT (fp32):  X = [Xr | Xi] -> evict to bf16 padded slots ----
            Xp = ps_x.tile([F, 2 * T], FP32, tag="Xp")
            nc.tensor.matmul(Xp[:, :], Fc[:, 0:F], frm[:, T:3 * T], start=True, stop=False)
            nc.tensor.matmul(Xp[:, :], Fc[:, F:2 * F], frm[:, 0:2 * T], start=False, stop=True)
            Xsb = wpool.tile([F, 2 * TP], BF16, tag="Xsb")
            Xsv = Xsb[:, :].rearrange("p (c t) -> p c t", c=2)
            Xpv = Xp[:, :].rearrange("p (c t) -> p c t", c=2)
            nc.scalar.activation(Xsv[:, :, 0:T], Xpv[:, :, 0:T],
                                 mybir.ActivationFunctionType.Copy)

            # ---- per-chunk tiles ----
            Rsb, Csb, Usb, Vsb = [], [], [], []
            for c in range(NCH):
                Rsb.append(wpool.tile([F, CHJ * 2 * TP], BF16, tag=f"Rsb{c}", name=f"Rsb{c}"))
                Csb.append(wpool.tile([F, CHJ * 3 * TP], BF16, tag=f"Csb{c}", name=f"Csb{c}"))
                Usb.append(wpool.tile([F, CHJ * 2 * TP], BF16, tag=f"Usb{c}", name=f"Usb{c}"))
                Vsb.append(wpool.tile([F, CHJ * 3 * TP], BF16, tag=f"Vsb{c}", name=f"Vsb{c}"))
            sPR = wpool.tile([F, CHJ * TP], BF16, tag="sPR")
            sPI = wpool.tile([F, CHJ * TP], BF16, tag="sPI")
            sA = wpool.tile([F, CHJ * TP], BF16, tag="sA")
            sB = wpool.tile([F, CHJ * TP], BF16, tag="sB")
            sC = wpool.tile([F, CHJ * TP], BF16, tag="sC")
            sD = wpool.tile([F, CHJ * TP], BF16, tag="sD")

            Dp = ps_d.tile([F, 2 * T], FP32, tag="Dp")

            TT = nc.vector.tensor_tensor
            TG = nc.gpsimd.tensor_tensor
            MUL = mybir.AluOpType.mult
            ADD = mybir.AluOpType.add
            SUB = mybir.AluOpType.subtract

            for c in range(NCH):
                # R build: permutation matmuls (bf16), 5 per bank, ACT-evict
                for bk in range(CHJ // PBK):
                    Rp = ps_r.tile([F, PBK * 2 * T], FP32, tag="Rp")
                    Xrhs = bass.AP(tensor=Xsb[:, :].tensor, offset=Xsb[:, :].offset,
                                   ap=[[2 * TP, F], [TP, 2], [1, T]])
                    for s in range(PBK):
                        j = c * CHJ + bk * PBK + s
                        nc.tensor.matmul(Rp[:, s * 2 * T:(s + 1) * 2 * T],
                                         Ssb[:, j * F:(j + 1) * F],
                                         Xrhs, start=True, stop=True)
                    # evict psum (5j x [Ur51|Ui51]) -> bf16 padded slots
                    nc.scalar.activation(
                        bass.AP(tensor=Rsb[c][:, :].tensor,
                                offset=Rsb[c][:, :].offset + bk * PBK * 2 * TP,
                                ap=[[CHJ * 2 * TP, F], [2 * TP, PBK], [TP, 2], [1, T]]),
                        Rp[:, :].rearrange("p (s c t) -> p s c t", s=PBK, c=2),
                        mybir.ActivationFunctionType.Copy)

                # full-slot (52-wide, pad included) dense views for fast DVE modes
                Rv = Rsb[c][:, :].rearrange("p (j s) -> p j s", j=CHJ)
                Rr = Rv[:, :, 0:TP]
                Ri = Rv[:, :, TP:2 * TP]
                Xbr = Xsb[:, None, 0:TP].to_broadcast([F, CHJ, TP])
                Xbi = Xsb[:, None, TP:2 * TP].to_broadcast([F, CHJ, TP])
                vPR = sPR[:, :].rearrange("p (j t) -> p j t", j=CHJ)
                vPI = sPI[:, :].rearrange("p (j t) -> p j t", j=CHJ)
                vA = sA[:, :].rearrange("p (j t) -> p j t", j=CHJ)
                vB = sB[:, :].rearrange("p (j t) -> p j t", j=CHJ)
                vC = sC[:, :].rearrange("p (j t) -> p j t", j=CHJ)
                vD = sD[:, :].rearrange("p (j t) -> p j t", j=CHJ)

                # C_pre = X * conj(R)
                TT(vA, Xbr, Rr, MUL)
                TT(vB, Xbi, Ri, MUL)
                TT(vPR, vA, vB, ADD)
                TG(vC, Xbi, Rr, MUL)
                TG(vD, Xbr, Ri, MUL)
                TG(vPI, vC, vD, SUB)

                # C = C_pre + roll_t(C_pre), slots [CiN | Cr | Ci]
                Cv = Csb[c][:, :].rearrange("p (j s) -> p j s", j=CHJ)
                TT(Cv[:, :, TP + 1:TP + T], vPR[:, :, 1:T], vPR[:, :, 0:T - 1], ADD)
                TT(Cv[:, :, TP:TP + 1], vPR[:, :, 0:1], vPR[:, :, T - 1:T], ADD)
                TG(Cv[:, :, 2 * TP + 1:2 * TP + T], vPI[:, :, 1:T], vPI[:, :, 0:T - 1], ADD)
                TG(Cv[:, :, 2 * TP:2 * TP + 1], vPI[:, :, 0:1], vPI[:, :, T - 1:T], ADD)
                nc.scalar.activation(Cv[:, :, 0:T], Cv[:, :, 2 * TP:2 * TP + T],
                                     mybir.ActivationFunctionType.Copy, scale=-1.0)

                # stage-1: U = M.T @ C -> [Ur | Ui] (psum packed), ACT-evict
                for bk in range(CHJ // PBK):
                    Up = ps_u.tile([F, PBK * 2 * T], FP32, tag="Up")
                    for s in range(PBK):
                        jj = bk * PBK + s
                        j = c * CHJ + jj
                        rhs1 = bass.AP(tensor=Csb[c][:, :].tensor,
                                       offset=Csb[c][:, :].offset + jj * 3 * TP + TP,
                                       ap=[[3 * CHJ * TP, F], [TP, 2], [1, T]])
                        rhs2 = bass.AP(tensor=Csb[c][:, :].tensor,
                                       offset=Csb[c][:, :].offset + jj * 3 * TP,
                                       ap=[[3 * CHJ * TP, F], [TP, 2], [1, T]])
                        nc.tensor.matmul(Up[:, s * 2 * T:(s + 1) * 2 * T],
                                         Msb[:, (2 * j) * F:(2 * j + 1) * F],
                                         rhs1, start=True, stop=False)
                        nc.tensor.matmul(Up[:, s * 2 * T:(s + 1) * 2 * T],
                                         Msb[:, (2 * j + 1) * F:(2 * j + 2) * F],
                                         rhs2, start=False, stop=True)
                    nc.scalar.activation(
                        bass.AP(tensor=Usb[c][:, :].tensor,
                                offset=Usb[c][:, :].offset + bk * PBK * 2 * TP,
                                ap=[[CHJ * 2 * TP, F], [2 * TP, PBK], [TP, 2], [1, T]]),
                        Up[:, :].rearrange("p (s c t) -> p s c t", s=PBK, c=2),
                        mybir.ActivationFunctionType.Copy)

                # stage-2: V = U * R, slots [ViN | Vr | Vi]
                Uv = Usb[c][:, :].rearrange("p (j s) -> p j s", j=CHJ)
                Ur = Uv[:, :, 0:TP]
                Ui = Uv[:, :, TP:2 * TP]
                Vv = Vsb[c][:, :].rearrange("p (j s) -> p j s", j=CHJ)
                TT(vA, Ur, Rr, MUL)
                TT(vB, Ui, Ri, MUL)
                TT(Vv[:, :, TP:2 * TP], vA, vB, SUB)
                TG(vC, Ur, Ri, MUL)
                TG(vD, Ui, Rr, MUL)
                TG(Vv[:, :, 2 * TP:3 * TP], vC, vD, ADD)
                nc.scalar.activation(Vv[:, :, 0:TP], Vv[:, :, 2 * TP:3 * TP],
                                     mybir.ActivationFunctionType.Copy, scale=-1.0)

                # final: D += Gr.T@[Vr|Vi] (all j), then Gi.T@[ViN|Vr] (all j)
                for gpass in range(2):
                    for jj in range(CHJ):
                        j = c * CHJ + jj
                        off = jj * 3 * TP + (TP if gpass == 0 else 0)
                        rhs = bass.AP(tensor=Vsb[c][:, :].tensor,
                                      offset=Vsb[c][:, :].offset + off,
                                      ap=[[3 * CHJ * TP, F], [TP, 2], [1, T]])
                        nc.tensor.matmul(Dp[:, :], Gc[:, gpass * F:(gpass + 1) * F],
                                         rhs, start=(c == 0 and gpass == 0 and jj == 0),
                                         stop=(c == NCH - 1 and gpass == 1 and jj == CHJ - 1))

            # ---- overlap-add + scale (fp32) ----
            Dsb = wpool.tile([F, 2 * T], FP32, tag="Dsb")
            nc.scalar.activation(Dsb[:, :], Dp[:, :],
                                 mybir.ActivationFunctionType.Copy)
            S2 = wpool.tile([HOP, 2 * T], FP32, tag="S2")
            nc.sync.dma_start(S2[:, :], Dsb[HOP:F, :])
            Y = wpool.tile([HOP, 2 * 52], FP32, tag="Y")
            S1v = Dsb[0:HOP, :].rearrange("p (c t) -> p c t", c=2)
            S2v = S2[:, :].rearrange("p (c t) -> p c t", c=2)
            Yv = Y[:, :].rearrange("p (c t) -> p c t", c=2)
            TT(Yv[:, :, 1:T], S1v[:, :, 1:T], S2v[:, :, 0:T - 1], ADD)
            nc.scalar.activation(Yv[:, :, 0:1], S1v[:, :, 0:1],
                                 mybir.ActivationFunctionType.Copy)
            nc.scalar.activation(Yv[:, :, T:52], S2v[:, :, T - 1:T],
                                 mybir.ActivationFunctionType.Copy)
            TT(Yv, Yv, sv[:, None, :].to_broadcast([HOP, 2, 52]), MUL)

            for p in range(2):
                nc.sync.dma_start(
                    bass.AP(tensor=yv[:, :].tensor, offset=p * L,
                            ap=[[1, HOP], [HOP, 52]]),
                    Y[:, p * 52:(p + 1) * 52],
                )
    return nc


# ---------------- host side ----------------

def _host_consts():
    W, G = _dft_consts()
    fr_c = np.concatenate([W.real, W.imag], axis=1).astype(np.float32)
    gr_c = np.concatenate([G.real, G.imag], axis=1).astype(bfloat16)
    cov = np.zeros(L)
    idx = (np.arange(T)[:, None] * HOP + np.arange(F)[None, :]).reshape(-1)
    np.add.at(cov, idx, 1.0)
    cov = np.where(cov > 0, cov, 1.0)
    return fr_c, gr_c, cov


def _smat_for(n2_list):
    S = np.zeros((NJ, F, F), np.float32)
    g = np.arange(F)
    for j, n2 in enumerate(n2_list):
        S[j, (g - n2) % F, g] = 1.0
    return np.ascontiguousarray(S.transpose(1, 0, 2).reshape(F, NJ * F)).astype(bfloat16)


def _mst_for(n2_list, w2):
    Ms = np.zeros((NJ, 2, F, F), np.float32)
    g = np.arange(F)[:, None]
    f = np.arange(F)[None, :]
    n1 = ((f - g + 20) % F) - 20
    valid = (n1 >= -20) & (n1 <= 19)
    n1c = np.clip(n1 + 20, 0, 39)
    for j, n2 in enumerate(n2_list):
        col = w2[:, n2 + 20]
        Ms[j, 0] = np.where(valid, col.real[n1c], 0.0)
        Ms[j, 1] = np.where(valid, col.imag[n1c], 0.0)
    return np.ascontiguousarray(
        Ms.transpose(2, 0, 1, 3).reshape(F, NJ * 2 * F)).astype(bfloat16)


def make_in_maps(x_real, x_imag, task_info, w_real, w_imag):
    fr_c, gr_c, cov = _host_consts()
    b, _, m = x_real.shape
    P = np.power(10.0, task_info[:, 0] / 10.0) / m
    w2 = (np.asarray(w_real) + 1j * np.asarray(w_imag)).reshape(40, 40)
    smats = [_smat_for(nl) for nl in N2_LISTS]
    msts = [_mst_for(nl, w2) for nl in N2_LISTS]

    svs = []
    tp = np.arange(52)[None, :]
    tau = np.arange(HOP)[:, None]
    l = HOP * tp + tau  # always < L
    for bb in range(b):
        svs.append((P[bb] / cov[l]).astype(np.float32))

    in_maps = []
    shards = []
    for bb in range(b):
        for mm in range(m):
            for h in range(2):
                xvv = np.stack([x_real[bb, :, mm], x_imag[bb, :, mm]]).astype(np.float32)
                in_maps.append({
                    "xv": xvv,
                    "fr_c": fr_c,
                    "gr_c": gr_c,
                    "smat": smats[h],
                    "mst": msts[h],
                    "svec": svs[bb],
                })
                shards.append((bb, mm, h))
    return in_maps, shards, P, cov


_NC_CACHE = {}


def kernel(x_real, x_imag, task_info, w_real, w_imag, b_real, b_imag):
    x_real = np.asarray(x_real)
    x_imag = np.asarray(x_imag)
    task_info = np.asarray(task_info)
    b, Lx, m = x_real.shape
    assert (b, Lx, m) == (2, L, 2)

    if "nc" not in _NC_CACHE:
        nc_ = build_program(debug=False)
        nc_.compile()
        _NC_CACHE["nc"] = nc_
    nc = _NC_CACHE["nc"]

    in_maps, shards, P, cov = make_in_maps(x_real, x_imag, task_info, w_real, w_imag)
    from concourse.bass_utils import run_bass_kernel_spmd
    res = run_bass_kernel_spmd(nc, in_maps, list(range(8))).results

    x = (x_real + 1j * x_imag).astype(np.complex64)
    out = x.copy()
    bias = complex(np.asarray(b_real)[0], np.asarray(b_imag)[0])
    bias_sig = np.zeros(L, np.complex64)
    bias_sig[np.arange(T) * HOP] = bias
    bias_sig /= cov
    for i, (bb, mm, h) in enumerate(shards):
        yvv = res[i]["yv"]
        out[bb, :, mm] += yvv[0] + 1j * yvv[1]
    for bb in range(b):
        for mm in range(m):
            out[bb, :, mm] += (P[bb] * bias_sig).astype(np.complex64)
    return out[:, 20:L - 20, :]


# revision 18
# speedup vs baseline: 1.9670x; 1.3566x over previous
"""Trainium2 Bass kernel for nn_EqStftPBC (STFT perturbation-based compensation).

Per (batch b, mode m):
  X = STFT(x); C_n2 = X*conj(roll(X,n2)) + prev-frame; U_n2 = circulant(w[:,n2]);
  V_n2 = U_n2 * roll(X,n2); delta = sum_n2 V_n2; out = x + ISTFT(delta)*P (+bias)

8 cores = (b x m x n2-half), uniform SPMD program; per-core variation only in
input data (permutation stack S, circulant stack M).  Device layout: [freq(80)
partitions, time free].  STFT fp32, rest bf16 (PSUM fp32).
"""

import numpy as np
from ml_dtypes import bfloat16

import concourse.bass as bass
import concourse.bacc as bacc
import concourse.mybir as mybir
import concourse.tile as tile

F = 80
T = 51
TP = 52          # padded slot stride
HOP = 40
L = 2080
NJ = 20
NCH = 2
CHJ = NJ // NCH
PBK = 5          # stage-1/R psum outputs per bank
GJ = 5           # j per merged G-matmul (N = GJ*102 <= 512)
FP32 = mybir.dt.float32
BF16 = mybir.dt.bfloat16

N2_LISTS = [list(range(19, -1, -1)), list(range(-1, -21, -1))]


def _dft_consts():
    j = np.arange(F)
    W = np.exp(-2j * np.pi * np.outer(j, j) / F)
    G = np.exp(+2j * np.pi * np.outer(j, j) / F) / F
    return W, G


def build_program(debug=False):
    nc = bacc.Bacc("TRN2", target_bir_lowering=False, debug=debug)

    # xf = [fiN | fr | fi] frames, pre-framed on host (pure reshape)
    xf = nc.dram_tensor("xf", [F, 3 * T], FP32, kind="ExternalInput")
    fr_c = nc.dram_tensor("fr_c", [F, 2 * F], FP32, kind="ExternalInput")
    gr_c = nc.dram_tensor("gr_c", [F, 2 * F], BF16, kind="ExternalInput")
    smat = nc.dram_tensor("smat", [F, NJ * F], BF16, kind="ExternalInput")
    mst = nc.dram_tensor("mst", [F, NJ * 2 * F], BF16, kind="ExternalInput")
    svec = nc.dram_tensor("svec", [HOP, 52], FP32, kind="ExternalInput")
    yv = nc.dram_tensor("yv", [HOP, 2 * 52], FP32, kind="ExternalOutput")

    with tile.TileContext(nc) as tc:
        with (
            tc.tile_pool(name="const", bufs=1) as cpool,
            tc.tile_pool(name="work", bufs=1) as wpool,
            tc.tile_pool(name="ps_x", bufs=1, space="PSUM") as ps_x,
            tc.tile_pool(name="ps_r", bufs=2, space="PSUM") as ps_r,
            tc.tile_pool(name="ps_u", bufs=2, space="PSUM") as ps_u,
            tc.tile_pool(name="ps_d", bufs=1, space="PSUM") as ps_d,
        ):
            frm = wpool.tile([F, 3 * T], FP32, tag="frm")
            nc.sync.dma_start(frm[:, :], xf[:, :])
            Fc = cpool.tile([F, 2 * F], FP32, tag="Fc")
            nc.sync.dma_start(Fc[:, :], fr_c[:, :])
            Ssb = cpool.tile([F, NJ * F], BF16, tag="Ssb")
            nc.sync.dma_start(Ssb[:, :], smat[:, :])
            Msb = cpool.tile([F, NJ * 2 * F], BF16, tag="Msb")
            for c in range(NCH):
                nc.gpsimd.dma_start(Msb[:, c * CHJ * 2 * F:(c + 1) * CHJ * 2 * F],
                                    mst[:, c * CHJ * 2 * F:(c + 1) * CHJ * 2 * F])
            Gc = cpool.tile([F, 2 * F], BF16, tag="Gc")
            nc.gpsimd.dma_start(Gc[:, :], gr_c[:, :])
            sv = cpool.tile([HOP, 52], FP32, tag="sv")
            nc.gpsimd.dma_start(sv[:, :], svec[:, :])

            # ---- STFT (fp32) -> X bf16 [Xr(52) | Xi(52)] ----
            Xp = ps_x.tile([F, 2 * T], FP32, tag="Xp")
            nc.tensor.matmul(Xp[:, :], Fc[:, 0:F], frm[:, T:3 * T], start=True, stop=False)
            nc.tensor.matmul(Xp[:, :], Fc[:, F:2 * F], frm[:, 0:2 * T], start=False, stop=True)
            Xsb = wpool.tile([F, 2 * TP], BF16, tag="Xsb")
            Xsv = Xsb[:, :].rearrange("p (c t) -> p c t", c=2)
            nc.scalar.activation(Xsv[:, :, 0:T],
                                 Xp[:, :].rearrange("p (c t) -> p c t", c=2),
                                 mybir.ActivationFunctionType.Copy)
            Xrhs = bass.AP(tensor=Xsb[:, :].tensor, offset=Xsb[:, :].offset,
                           ap=[[2 * TP, F], [TP, 2], [1, T]])

            # plane-major per-chunk stacks: R/U = [r-block | i-block], blocks CHJ*TP
            # C/V = [negi-block | r-block | i-block]
            BL = CHJ * TP
            Rsb, Csb, Usb, Vsb = [], [], [], []
            for c in range(NCH):
                Rsb.append(wpool.tile([F, 2 * BL], BF16, tag=f"Rsb{c}", name=f"Rsb{c}"))
                Csb.append(wpool.tile([F, 3 * BL], BF16, tag=f"Csb{c}", name=f"Csb{c}"))
                Usb.append(wpool.tile([F, 2 * BL], BF16, tag=f"Usb{c}", name=f"Usb{c}"))
                Vsb.append(wpool.tile([F, 3 * BL], BF16, tag=f"Vsb{c}", name=f"Vsb{c}"))
            sA = wpool.tile([F, BL], BF16, tag="sA")
            sB = wpool.tile([F, BL], BF16, tag="sB")
            sC = wpool.tile([F, BL], BF16, tag="sC")
            sD = wpool.tile([F, BL], BF16, tag="sD")
            sPR = wpool.tile([F, BL], BF16, tag="sPR")
            sPI = wpool.tile([F, BL], BF16, tag="sPI")

            Dp = ps_d.tile([F, GJ * 2 * T], FP32, tag="Dp")  # 5 accumulated [dr|di] pairs

            TT = nc.vector.tensor_tensor
            TG = nc.gpsimd.tensor_tensor
            MUL = mybir.AluOpType.mult
            ADD = mybir.AluOpType.add
            SUB = mybir.AluOpType.subtract
            CPY = mybir.ActivationFunctionType.Copy

            for c in range(NCH):
                Rc, Cc, Uc, Vc = Rsb[c], Csb[c], Usb[c], Vsb[c]
                # ---- R: permutation matmuls, PBK per bank, plane-major evict ----
                for bk in range(CHJ // PBK):
                    Rp = ps_r.tile([F, PBK * 2 * T], FP32, tag="Rp")
                    for s in range(PBK):
                        j = c * CHJ + bk * PBK + s
                        nc.tensor.matmul(Rp[:, s * 2 * T:(s + 1) * 2 * T],
                                         Ssb[:, j * F:(j + 1) * F],
                                         Xrhs, start=True, stop=True)
                    # psum [s, c2, t] -> Rsb [c2-block, (bk*PBK+s)*TP + t]
                    dst = bass.AP(tensor=Rc[:, :].tensor,
                                  offset=Rc[:, :].offset + bk * PBK * TP,
                                  ap=[[2 * BL, F], [TP, PBK], [BL, 2], [1, T]])
                    nc.scalar.activation(
                        dst, Rp[:, :].rearrange("p (s c2 t) -> p s c2 t", s=PBK, c2=2),
                        CPY)

                Rr = Rc[:, 0:BL].rearrange("p (j t) -> p j t", j=CHJ)
                Ri = Rc[:, BL:2 * BL].rearrange("p (j t) -> p j t", j=CHJ)
                Xbr = Xsb[:, None, 0:TP].to_broadcast([F, CHJ, TP])
                Xbi = Xsb[:, None, TP:2 * TP].to_broadcast([F, CHJ, TP])
                vA = sA[:, :].rearrange("p (j t) -> p j t", j=CHJ)
                vB = sB[:, :].rearrange("p (j t) -> p j t", j=CHJ)
                vC = sC[:, :].rearrange("p (j t) -> p j t", j=CHJ)
                vD = sD[:, :].rearrange("p (j t) -> p j t", j=CHJ)
                vPR = sPR[:, :].rearrange("p (j t) -> p j t", j=CHJ)
                vPI = sPI[:, :].rearrange("p (j t) -> p j t", j=CHJ)

                # ---- C_pre = X * conj(R) ----
                TT(vA, Xbr, Rr, MUL)
                TT(vB, Xbi, Ri, MUL)
                TT(vPR, vA, vB, ADD)
                TG(vC, Xbi, Rr, MUL)
                TG(vD, Xbr, Ri, MUL)
                TG(vPI, vC, vD, SUB)

                # ---- C = C_pre + roll_t;  blocks [CiN | Cr | Ci] ----
                CrB = Cc[:, BL:2 * BL].rearrange("p (j t) -> p j t", j=CHJ)
                CiB = Cc[:, 2 * BL:3 * BL].rearrange("p (j t) -> p j t", j=CHJ)
                TT(CrB[:, :, 1:T], vPR[:, :, 1:T], vPR[:, :, 0:T - 1], ADD)
                TT(CrB[:, :, 0:1], vPR[:, :, 0:1], vPR[:, :, T - 1:T], ADD)
                TG(CiB[:, :, 1:T], vPI[:, :, 1:T], vPI[:, :, 0:T - 1], ADD)
                TG(CiB[:, :, 0:1], vPI[:, :, 0:1], vPI[:, :, T - 1:T], ADD)
                nc.scalar.activation(Cc[:, 0:BL], Cc[:, 2 * BL:3 * BL], CPY, scale=-1.0)

                # ---- stage-1: U_j = Mr.T@[Cr|Ci] + Mi.T@[CiN|Cr] ----
                for bk in range(CHJ // PBK):
                    Up = ps_u.tile([F, PBK * 2 * T], FP32, tag="Up")
                    for s in range(PBK):
                        jj = bk * PBK + s
                        j = c * CHJ + jj
                        rhs1 = bass.AP(tensor=Cc[:, :].tensor,
                                       offset=Cc[:, :].offset + BL + jj * TP,
                                       ap=[[3 * BL, F], [BL, 2], [1, T]])
                        rhs2 = bass.AP(tensor=Cc[:, :].tensor,
                                       offset=Cc[:, :].offset + jj * TP,
                                       ap=[[3 * BL, F], [BL, 2], [1, T]])
                        nc.tensor.matmul(Up[:, s * 2 * T:(s + 1) * 2 * T],
                                         Msb[:, (2 * j) * F:(2 * j + 1) * F],
                                         rhs1, start=True, stop=False)
                        nc.tensor.matmul(Up[:, s * 2 * T:(s + 1) * 2 * T],
                                         Msb[:, (2 * j + 1) * F:(2 * j + 2) * F],
                                         rhs2, start=False, stop=True)
                    dst = bass.AP(tensor=Uc[:, :].tensor,
                                  offset=Uc[:, :].offset + bk * PBK * TP,
                                  ap=[[2 * BL, F], [TP, PBK], [BL, 2], [1, T]])
                    nc.scalar.activation(
                        dst, Up[:, :].rearrange("p (s c2 t) -> p s c2 t", s=PBK, c2=2),
                        CPY)

                # ---- stage-2: V = U * R;  blocks [ViN | Vr | Vi] ----
                Ur = Uc[:, 0:BL].rearrange("p (j t) -> p j t", j=CHJ)
                Ui = Uc[:, BL:2 * BL].rearrange("p (j t) -> p j t", j=CHJ)
                TT(vA, Ur, Rr, MUL)
                TT(vB, Ui, Ri, MUL)
                TT(Vc[:, BL:2 * BL], sA[:, :], sB[:, :], SUB)
                TG(vC, Ur, Ri, MUL)
                TG(vD, Ui, Rr, MUL)
                TG(Vc[:, 2 * BL:3 * BL], sC[:, :], sD[:, :], ADD)
                nc.scalar.activation(Vc[:, 0:BL], Vc[:, 2 * BL:3 * BL], CPY, scale=-1.0)

                # ---- merged G-matmuls: accumulate into 5 [dr|di] pairs ----
                for gpass in range(2):
                    for h in range(CHJ // GJ):
                        base = (BL if gpass == 0 else 0) + h * GJ * TP
                        rhs = bass.AP(tensor=Vc[:, :].tensor,
                                      offset=Vc[:, :].offset + base,
                                      ap=[[3 * BL, F], [TP, GJ], [BL, 2], [1, T]])
                        nc.tensor.matmul(
                            Dp[:, :].rearrange("p (s c2 t) -> p s c2 t", s=GJ, c2=2),
                            Gc[:, gpass * F:(gpass + 1) * F], rhs,
                            start=(c == 0 and gpass == 0 and h == 0),
                            stop=(c == NCH - 1 and gpass == 1 and h == CHJ // GJ - 1))

            # ---- reduce 5 pairs + overlap-add + scale (fp32) ----
            D5 = wpool.tile([F, GJ * 2 * T], FP32, tag="D5")
            nc.scalar.activation(D5[:, :], Dp[:, :], CPY)
            tE = wpool.tile([F, 4 * T], FP32, tag="tE")
            TT(tE[:, :], D5[:, 0:4 * T], D5[:, 4 * T:8 * T], ADD)       # p0+p2, p1+p3
            tF = wpool.tile([F, 2 * T], FP32, tag="tF")
            TT(tF[:, :], tE[:, 0:2 * T], tE[:, 2 * T:4 * T], ADD)
            Dsb = wpool.tile([F, 2 * T], FP32, tag="Dsb")
            TT(Dsb[:, :], tF[:, :], D5[:, 8 * T:10 * T], ADD)

            S2 = wpool.tile([HOP, 2 * T], FP32, tag="S2")
            nc.sync.dma_start(S2[:, :], Dsb[HOP:F, :])
            Y = wpool.tile([HOP, 2 * 52], FP32, tag="Y")
            S1v = Dsb[0:HOP, :].rearrange("p (c t) -> p c t", c=2)
            S2v = S2[:, :].rearrange("p (c t) -> p c t", c=2)
            Yv = Y[:, :].rearrange("p (c t) -> p c t", c=2)
            TT(Yv[:, :, 1:T], S1v[:, :, 1:T], S2v[:, :, 0:T - 1], ADD)
            nc.scalar.activation(Yv[:, :, 0:1], S1v[:, :, 0:1], CPY)
            nc.scalar.activation(Yv[:, :, T:52], S2v[:, :, T - 1:T], CPY)
            TT(Yv, Yv, sv[:, None, :].to_broadcast([HOP, 2, 52]), MUL)
            nc.sync.dma_start(yv[:, :], Y[:, :])
    return nc


# ---------------- host side ----------------

def _host_consts():
    W, G = _dft_consts()
    fr_c = np.concatenate([W.real, W.imag], axis=1).astype(np.float32)
    gr_c = np.concatenate([G.real, G.imag], axis=1).astype(bfloat16)
    cov = np.zeros(L)
    idx = (np.arange(T)[:, None] * HOP + np.arange(F)[None, :]).reshape(-1)
    np.add.at(cov, idx, 1.0)
    cov = np.where(cov > 0, cov, 1.0)
    return fr_c, gr_c, cov


def _smat_for(n2_list):
    S = np.zeros((NJ, F, F), np.float32)
    g = np.arange(F)
    for j, n2 in enumerate(n2_list):
        S[j, (g - n2) % F, g] = 1.0
    return np.ascontiguousarray(S.transpose(1, 0, 2).reshape(F, NJ * F)).astype(bfloat16)


def _mst_for(n2_list, w2):
    Ms = np.zeros((NJ, 2, F, F), np.float32)
    g = np.arange(F)[:, None]
    f = np.arange(F)[None, :]
    n1 = ((f - g + 20) % F) - 20
    valid = (n1 >= -20) & (n1 <= 19)
    n1c = np.clip(n1 + 20, 0, 39)
    for j, n2 in enumerate(n2_list):
        col = w2[:, n2 + 20]
        Ms[j, 0] = np.where(valid, col.real[n1c], 0.0)
        Ms[j, 1] = np.where(valid, col.imag[n1c], 0.0)
    return np.ascontiguousarray(
        Ms.transpose(2, 0, 1, 3).reshape(F, NJ * 2 * F)).astype(bfloat16)


def _frame(sig):
    idx = np.arange(T)[None, :] * HOP + np.arange(F)[:, None]   # [j, t]
    return sig[idx].astype(np.float32)


def make_in_maps(x_real, x_imag, task_info, w_real, w_imag):
    fr_c, gr_c, cov = _host_consts()
    b, _, m = x_real.shape
    P = np.power(10.0, task_info[:, 0] / 10.0) / m
    w2 = (np.asarray(w_real) + 1j * np.asarray(w_imag)).reshape(40, 40)
    smats = [_smat_for(nl) for nl in N2_LISTS]
    msts = [_mst_for(nl, w2) for nl in N2_LISTS]

    tp = np.arange(52)[None, :]
    tau = np.arange(HOP)[:, None]
    l = HOP * tp + tau
    svs = [(P[bb] / cov[l]).astype(np.float32) for bb in range(b)]

    in_maps, shards = [], []
    for bb in range(b):
        for mm in range(m):
            fr_ = _frame(x_real[bb, :, mm])
            fi_ = _frame(x_imag[bb, :, mm])
            xfv = np.concatenate([-fi_, fr_, fi_], axis=1).astype(np.float32)
            for h in range(2):
                in_maps.append({
                    "xf": xfv,
                    "fr_c": fr_c,
                    "gr_c": gr_c,
                    "smat": smats[h],
                    "mst": msts[h],
                    "svec": svs[bb],
                })
                shards.append((bb, mm, h))
    return in_maps, shards, P, cov


_NC_CACHE = {}


def kernel(x_real, x_imag, task_info, w_real, w_imag, b_real, b_imag):
    x_real = np.asarray(x_real)
    x_imag = np.asarray(x_imag)
    task_info = np.asarray(task_info)
    b, Lx, m = x_real.shape
    assert (b, Lx, m) == (2, L, 2)

    if "nc" not in _NC_CACHE:
        nc_ = build_program(debug=False)
        nc_.compile()
        _NC_CACHE["nc"] = nc_
    nc = _NC_CACHE["nc"]

    in_maps, shards, P, cov = make_in_maps(x_real, x_imag, task_info, w_real, w_imag)
    from concourse.bass_utils import run_bass_kernel_spmd
    res = run_bass_kernel_spmd(nc, in_maps, list(range(8))).results

    x = (x_real + 1j * x_imag).astype(np.complex64)
    out = x.copy()
    bias = complex(np.asarray(b_real)[0], np.asarray(b_imag)[0])
    bias_sig = np.zeros(L, np.complex64)
    bias_sig[np.arange(T) * HOP] = bias
    bias_sig /= cov
    for i, (bb, mm, h) in enumerate(shards):
        yvv = res[i]["yv"]          # [40, 104] = [tau, (yr(52) | yi(52))]
        yr = yvv[:, 0:52].T.ravel()[:L]
        yi = yvv[:, 52:104].T.ravel()[:L]
        out[bb, :, mm] += yr + 1j * yi
    for bb in range(b):
        for mm in range(m):
            out[bb, :, mm] += (P[bb] * bias_sig).astype(np.complex64)
    return out[:, 20:L - 20, :]


# revision 21
# speedup vs baseline: 1.9689x; 1.0010x over previous
"""Trainium2 Bass kernel for nn_EqStftPBC (STFT perturbation-based compensation).

Per (batch b, mode m):
  X = STFT(x); C_n2 = X*conj(roll(X,n2)) + prev-frame; U_n2 = circulant(w[:,n2]);
  V_n2 = U_n2 * roll(X,n2); delta = sum_n2 V_n2; out = x + ISTFT(delta)*P (+bias)

8 cores = (b x m x n2-half), uniform SPMD program; per-core variation only in
input data (permutation stack S, circulant stack M).  Device layout: [freq(80)
partitions, time free].  STFT fp32, rest bf16 (PSUM fp32).
"""

import numpy as np
from ml_dtypes import bfloat16

import concourse.bass as bass
import concourse.bacc as bacc
import concourse.mybir as mybir
import concourse.tile as tile

F = 80
T = 51
TP = 52          # padded slot stride
HOP = 40
L = 2080
NJ = 20
NCH = 2
CHJ = NJ // NCH
PBK = 5          # stage-1/R psum outputs per bank
GJ = 5           # j per merged G-matmul (N = GJ*102 <= 512)
FP32 = mybir.dt.float32
BF16 = mybir.dt.bfloat16

N2_LISTS = [list(range(19, -1, -1)), list(range(-1, -21, -1))]


def _dft_consts():
    j = np.arange(F)
    W = np.exp(-2j * np.pi * np.outer(j, j) / F)
    G = np.exp(+2j * np.pi * np.outer(j, j) / F) / F
    return W, G


def build_program(debug=False):
    nc = bacc.Bacc("TRN2", target_bir_lowering=False, debug=debug)

    # xf = [fiN | fr | fi] frames, pre-framed on host (pure reshape)
    xf = nc.dram_tensor("xf", [F, 3 * T], FP32, kind="ExternalInput")
    fr_c = nc.dram_tensor("fr_c", [F, 2 * F], FP32, kind="ExternalInput")
    gr_c = nc.dram_tensor("gr_c", [F, 2 * F], BF16, kind="ExternalInput")
    smat = nc.dram_tensor("smat", [F, NJ * F], BF16, kind="ExternalInput")
    mst = nc.dram_tensor("mst", [F, NJ * 2 * F], BF16, kind="ExternalInput")
    svec = nc.dram_tensor("svec", [HOP, 52], FP32, kind="ExternalInput")
    yv = nc.dram_tensor("yv", [HOP, 2 * 52], FP32, kind="ExternalOutput")

    with tile.TileContext(nc) as tc:
        with (
            tc.tile_pool(name="const", bufs=1) as cpool,
            tc.tile_pool(name="work", bufs=1) as wpool,
            tc.tile_pool(name="ps_x", bufs=1, space="PSUM") as ps_x,
            tc.tile_pool(name="ps_r", bufs=2, space="PSUM") as ps_r,
            tc.tile_pool(name="ps_u", bufs=2, space="PSUM") as ps_u,
            tc.tile_pool(name="ps_d", bufs=1, space="PSUM") as ps_d,
        ):
            frm = wpool.tile([F, 3 * T], FP32, tag="frm")
            nc.sync.dma_start(frm[:, :], xf[:, :])
            Fc = cpool.tile([F, 2 * F], FP32, tag="Fc")
            nc.sync.dma_start(Fc[:, :], fr_c[:, :])
            Ssb = cpool.tile([F, NJ * F], BF16, tag="Ssb")
            for q in range(NJ // PBK):
                nc.sync.dma_start(Ssb[:, q * PBK * F:(q + 1) * PBK * F],
                                  smat[:, q * PBK * F:(q + 1) * PBK * F])
            Msb = cpool.tile([F, NJ * 2 * F], BF16, tag="Msb")
            for c in range(NCH):
                nc.gpsimd.dma_start(Msb[:, c * CHJ * 2 * F:(c + 1) * CHJ * 2 * F],
                                    mst[:, c * CHJ * 2 * F:(c + 1) * CHJ * 2 * F])
            Gc = cpool.tile([F, 2 * F], BF16, tag="Gc")
            nc.gpsimd.dma_start(Gc[:, :], gr_c[:, :])
            sv = cpool.tile([HOP, 52], FP32, tag="sv")
            nc.gpsimd.dma_start(sv[:, :], svec[:, :])

            # ---- STFT (fp32) -> X bf16 [Xr(52) | Xi(52)] ----
            Xp = ps_x.tile([F, 2 * T], FP32, tag="Xp")
            nc.tensor.matmul(Xp[:, :], Fc[:, 0:F], frm[:, T:3 * T], start=True, stop=False)
            nc.tensor.matmul(Xp[:, :], Fc[:, F:2 * F], frm[:, 0:2 * T], start=False, stop=True)
            Xsb = wpool.tile([F, 2 * TP], BF16, tag="Xsb")
            Xsv = Xsb[:, :].rearrange("p (c t) -> p c t", c=2)
            nc.scalar.activation(Xsv[:, :, 0:T],
                                 Xp[:, :].rearrange("p (c t) -> p c t", c=2),
                                 mybir.ActivationFunctionType.Copy)
            Xrhs = bass.AP(tensor=Xsb[:, :].tensor, offset=Xsb[:, :].offset,
                           ap=[[2 * TP, F], [TP, 2], [1, T]])

            # plane-major per-chunk stacks: R/U = [r-block | i-block], blocks CHJ*TP
            # C/V = [negi-block | r-block | i-block]
            BL = CHJ * TP
            Rsb, Csb, Usb, Vsb = [], [], [], []
            for c in range(NCH):
                Rsb.append(wpool.tile([F, 2 * BL], BF16, tag=f"Rsb{c}", name=f"Rsb{c}"))
                Csb.append(wpool.tile([F, 3 * BL], BF16, tag=f"Csb{c}", name=f"Csb{c}"))
                Usb.append(wpool.tile([F, 2 * BL], BF16, tag=f"Usb{c}", name=f"Usb{c}"))
                Vsb.append(wpool.tile([F, 3 * BL], BF16, tag=f"Vsb{c}", name=f"Vsb{c}"))
            sA = wpool.tile([F, BL], BF16, tag="sA")
            sB = wpool.tile([F, BL], BF16, tag="sB")
            sC = wpool.tile([F, BL], BF16, tag="sC")
            sD = wpool.tile([F, BL], BF16, tag="sD")
            sPR = wpool.tile([F, BL], BF16, tag="sPR")
            sPI = wpool.tile([F, BL], BF16, tag="sPI")

            Dp = ps_d.tile([F, GJ * 2 * T], FP32, tag="Dp")  # 5 accumulated [dr|di] pairs

            TT = nc.vector.tensor_tensor
            TG = nc.gpsimd.tensor_tensor
            MUL = mybir.AluOpType.mult
            ADD = mybir.AluOpType.add
            SUB = mybir.AluOpType.subtract
            CPY = mybir.ActivationFunctionType.Copy

            for c in range(NCH):
                Rc, Cc, Uc, Vc = Rsb[c], Csb[c], Usb[c], Vsb[c]
                # ---- R: permutation matmuls, PBK per bank, plane-major evict ----
                for bk in range(CHJ // PBK):
                    Rp = ps_r.tile([F, PBK * 2 * T], FP32, tag="Rp")
                    for s in range(PBK):
                        j = c * CHJ + bk * PBK + s
                        nc.tensor.matmul(Rp[:, s * 2 * T:(s + 1) * 2 * T],
                                         Ssb[:, j * F:(j + 1) * F],
                                         Xrhs, start=True, stop=True)
                    # psum [s, c2, t] -> Rsb [c2-block, (bk*PBK+s)*TP + t]
                    dst = bass.AP(tensor=Rc[:, :].tensor,
                                  offset=Rc[:, :].offset + bk * PBK * TP,
                                  ap=[[2 * BL, F], [TP, PBK], [BL, 2], [1, T]])
                    nc.scalar.activation(
                        dst, Rp[:, :].rearrange("p (s c2 t) -> p s c2 t", s=PBK, c2=2),
                        CPY)

                Rrf = Rc[:, 0:BL]
                Rif = Rc[:, BL:2 * BL]
                vPR = sPR[:, :].rearrange("p (j t) -> p j t", j=CHJ)
                vPI = sPI[:, :].rearrange("p (j t) -> p j t", j=CHJ)

                # ---- C_pre = X * conj(R)  (flat 2D ops; Xt = tiled X copies) ----
                if c == 0:
                    Xtr = wpool.tile([F, BL], BF16, tag="Xtr")
                    Xti = wpool.tile([F, BL], BF16, tag="Xti")
                    nc.scalar.activation(
                        Xtr[:, :].rearrange("p (j t) -> p j t", j=CHJ),
                        Xsb[:, None, 0:TP].to_broadcast([F, CHJ, TP]), CPY)
                    nc.scalar.activation(
                        Xti[:, :].rearrange("p (j t) -> p j t", j=CHJ),
                        Xsb[:, None, TP:2 * TP].to_broadcast([F, CHJ, TP]), CPY)
                TT(sA[:, :], Xtr[:, :], Rrf, MUL)
                TT(sB[:, :], Xti[:, :], Rif, MUL)
                TT(sPR[:, :], sA[:, :], sB[:, :], ADD)
                TG(sC[:, :], Xti[:, :], Rrf, MUL)
                TG(sD[:, :], Xtr[:, :], Rif, MUL)
                TG(sPI[:, :], sC[:, :], sD[:, :], SUB)

                # ---- C = C_pre + roll_t;  blocks [CiN | Cr | Ci] ----
                CrB = Cc[:, BL:2 * BL].rearrange("p (j t) -> p j t", j=CHJ)
                CiB = Cc[:, 2 * BL:3 * BL].rearrange("p (j t) -> p j t", j=CHJ)
                TT(CrB[:, :, 1:T], vPR[:, :, 1:T], vPR[:, :, 0:T - 1], ADD)
                TT(CrB[:, :, 0:1], vPR[:, :, 0:1], vPR[:, :, T - 1:T], ADD)
                TG(CiB[:, :, 1:T], vPI[:, :, 1:T], vPI[:, :, 0:T - 1], ADD)
                TG(CiB[:, :, 0:1], vPI[:, :, 0:1], vPI[:, :, T - 1:T], ADD)
                nc.scalar.activation(Cc[:, 0:BL], Cc[:, 2 * BL:3 * BL], CPY, scale=-1.0)

                # ---- stage-1: U_j = Mr.T@[Cr|Ci] + Mi.T@[CiN|Cr] ----
                for bk in range(CHJ // PBK):
                    Up = ps_u.tile([F, PBK * 2 * T], FP32, tag="Up")
                    for s in range(PBK):
                        jj = bk * PBK + s
                        j = c * CHJ + jj
                        rhs1 = bass.AP(tensor=Cc[:, :].tensor,
                                       offset=Cc[:, :].offset + BL + jj * TP,
                                       ap=[[3 * BL, F], [BL, 2], [1, T]])
                        rhs2 = bass.AP(tensor=Cc[:, :].tensor,
                                       offset=Cc[:, :].offset + jj * TP,
                                       ap=[[3 * BL, F], [BL, 2], [1, T]])
                        nc.tensor.matmul(Up[:, s * 2 * T:(s + 1) * 2 * T],
                                         Msb[:, (2 * j) * F:(2 * j + 1) * F],
                                         rhs1, start=True, stop=False)
                        nc.tensor.matmul(Up[:, s * 2 * T:(s + 1) * 2 * T],
                                         Msb[:, (2 * j + 1) * F:(2 * j + 2) * F],
                                         rhs2, start=False, stop=True)
                    dst = bass.AP(tensor=Uc[:, :].tensor,
                                  offset=Uc[:, :].offset + bk * PBK * TP,
                                  ap=[[2 * BL, F], [TP, PBK], [BL, 2], [1, T]])
                    nc.scalar.activation(
                        dst, Up[:, :].rearrange("p (s c2 t) -> p s c2 t", s=PBK, c2=2),
                        CPY)

                # ---- stage-2: V = U * R;  blocks [ViN | Vr | Vi]  (flat 2D) ----
                Urf = Uc[:, 0:BL]
                Uif = Uc[:, BL:2 * BL]
                TT(sA[:, :], Urf, Rrf, MUL)
                TT(sB[:, :], Uif, Rif, MUL)
                TT(Vc[:, BL:2 * BL], sA[:, :], sB[:, :], SUB)
                TG(sC[:, :], Urf, Rif, MUL)
                TG(sD[:, :], Uif, Rrf, MUL)
                TG(Vc[:, 2 * BL:3 * BL], sC[:, :], sD[:, :], ADD)
                nc.scalar.activation(Vc[:, 0:BL], Vc[:, 2 * BL:3 * BL], CPY, scale=-1.0)

                # ---- merged G-matmuls: accumulate into 5 [dr|di] pairs ----
                for gpass in range(2):
                    for h in range(CHJ // GJ):
                        base = (BL if gpass == 0 else 0) + h * GJ * TP
                        rhs = bass.AP(tensor=Vc[:, :].tensor,
                                      offset=Vc[:, :].offset + base,
                                      ap=[[3 * BL, F], [TP, GJ], [BL, 2], [1, T]])
                        nc.tensor.matmul(
                            Dp[:, :].rearrange("p (s c2 t) -> p s c2 t", s=GJ, c2=2),
                            Gc[:, gpass * F:(gpass + 1) * F], rhs,
                            start=(c == 0 and gpass == 0 and h == 0),
                            stop=(c == NCH - 1 and gpass == 1 and h == CHJ // GJ - 1))

            # ---- reduce 5 pairs + overlap-add + scale (fp32) ----
            D5 = wpool.tile([F, GJ * 2 * T], FP32, tag="D5")
            nc.scalar.activation(D5[:, :], Dp[:, :], CPY)
            tE = wpool.tile([F, 4 * T], FP32, tag="tE")
            TT(tE[:, :], D5[:, 0:4 * T], D5[:, 4 * T:8 * T], ADD)       # p0+p2, p1+p3
            tF = wpool.tile([F, 2 * T], FP32, tag="tF")
            TT(tF[:, :], tE[:, 0:2 * T], tE[:, 2 * T:4 * T], ADD)
            Dsb = wpool.tile([F, 2 * T], FP32, tag="Dsb")
            TT(Dsb[:, :], tF[:, :], D5[:, 8 * T:10 * T], ADD)

            S2 = wpool.tile([HOP, 2 * T], FP32, tag="S2")
            nc.sync.dma_start(S2[:, :], Dsb[HOP:F, :])
            Y = wpool.tile([HOP, 2 * 52], FP32, tag="Y")
            S1v = Dsb[0:HOP, :].rearrange("p (c t) -> p c t", c=2)
            S2v = S2[:, :].rearrange("p (c t) -> p c t", c=2)
            Yv = Y[:, :].rearrange("p (c t) -> p c t", c=2)
            TT(Yv[:, :, 1:T], S1v[:, :, 1:T], S2v[:, :, 0:T - 1], ADD)
            nc.scalar.activation(Yv[:, :, 0:1], S1v[:, :, 0:1], CPY)
            nc.scalar.activation(Yv[:, :, T:52], S2v[:, :, T - 1:T], CPY)
            TT(Yv, Yv, sv[:, None, :].to_broadcast([HOP, 2, 52]), MUL)
            nc.sync.dma_start(yv[:, :], Y[:, :])
    return nc


# ---------------- host side ----------------

def _host_consts():
    W, G = _dft_consts()
    fr_c = np.concatenate([W.real, W.imag], axis=1).astype(np.float32)
    gr_c = np.concatenate([G.real, G.imag], axis=1).astype(bfloat16)
    cov = np.zeros(L)
    idx = (np.arange(T)[:, None] * HOP + np.arange(F)[None, :]).reshape(-1)
    np.add.at(cov, idx, 1.0)
    cov = np.where(cov > 0, cov, 1.0)
    return fr_c, gr_c, cov


def _smat_for(n2_list):
    S = np.zeros((NJ, F, F), np.float32)
    g = np.arange(F)
    for j, n2 in enumerate(n2_list):
        S[j, (g - n2) % F, g] = 1.0
    return np.ascontiguousarray(S.transpose(1, 0, 2).reshape(F, NJ * F)).astype(bfloat16)


def _mst_for(n2_list, w2):
    Ms = np.zeros((NJ, 2, F, F), np.float32)
    g = np.arange(F)[:, None]
    f = np.arange(F)[None, :]
    n1 = ((f - g + 20) % F) - 20
    valid = (n1 >= -20) & (n1 <= 19)
    n1c = np.clip(n1 + 20, 0, 39)
    for j, n2 in enumerate(n2_list):
        col = w2[:, n2 + 20]
        Ms[j, 0] = np.where(valid, col.real[n1c], 0.0)
        Ms[j, 1] = np.where(valid, col.imag[n1c], 0.0)
    return np.ascontiguousarray(
        Ms.transpose(2, 0, 1, 3).reshape(F, NJ * 2 * F)).astype(bfloat16)


def _frame(sig):
    idx = np.arange(T)[None, :] * HOP + np.arange(F)[:, None]   # [j, t]
    return sig[idx].astype(np.float32)


def make_in_maps(x_real, x_imag, task_info, w_real, w_imag):
    fr_c, gr_c, cov = _host_consts()
    b, _, m = x_real.shape
    P = np.power(10.0, task_info[:, 0] / 10.0) / m
    w2 = (np.asarray(w_real) + 1j * np.asarray(w_imag)).reshape(40, 40)
    smats = [_smat_for(nl) for nl in N2_LISTS]
    msts = [_mst_for(nl, w2) for nl in N2_LISTS]

    tp = np.arange(52)[None, :]
    tau = np.arange(HOP)[:, None]
    l = HOP * tp + tau
    svs = [(P[bb] / cov[l]).astype(np.float32) for bb in range(b)]

    in_maps, shards = [], []
    for bb in range(b):
        for mm in range(m):
            fr_ = _frame(x_real[bb, :, mm])
            fi_ = _frame(x_imag[bb, :, mm])
            xfv = np.concatenate([-fi_, fr_, fi_], axis=1).astype(np.float32)
            for h in range(2):
                in_maps.append({
                    "xf": xfv,
                    "fr_c": fr_c,
                    "gr_c": gr_c,
                    "smat": smats[h],
                    "mst": msts[h],
                    "svec": svs[bb],
                })
                shards.append((bb, mm, h))
    return in_maps, shards, P, cov


_NC_CACHE = {}


def kernel(x_real, x_imag, task_info, w_real, w_imag, b_real, b_imag):
    x_real = np.asarray(x_real)
    x_imag = np.asarray(x_imag)
    task_info = np.asarray(task_info)
    b, Lx, m = x_real.shape
    assert (b, Lx, m) == (2, L, 2)

    if "nc" not in _NC_CACHE:
        nc_ = build_program(debug=False)
        nc_.compile()
        _NC_CACHE["nc"] = nc_
    nc = _NC_CACHE["nc"]

    in_maps, shards, P, cov = make_in_maps(x_real, x_imag, task_info, w_real, w_imag)
    from concourse.bass_utils import run_bass_kernel_spmd
    res = run_bass_kernel_spmd(nc, in_maps, list(range(8))).results

    x = (x_real + 1j * x_imag).astype(np.complex64)
    out = x.copy()
    bias = complex(np.asarray(b_real)[0], np.asarray(b_imag)[0])
    bias_sig = np.zeros(L, np.complex64)
    bias_sig[np.arange(T) * HOP] = bias
    bias_sig /= cov
    for i, (bb, mm, h) in enumerate(shards):
        yvv = res[i]["yv"]          # [40, 104] = [tau, (yr(52) | yi(52))]
        yr = yvv[:, 0:52].T.ravel()[:L]
        yi = yvv[:, 52:104].T.ravel()[:L]
        out[bb, :, mm] += yr + 1j * yi
    for bb in range(b):
        for mm in range(m):
            out[bb, :, mm] += (P[bb] * bias_sig).astype(np.complex64)
    return out[:, 20:L - 20, :]
